# revision 7
# baseline (speedup 1.0000x reference)
"""Trainium2 Bass kernel for MultiHeadFrequencyCrossAttention.

Math note: the reference computes, per (batch, head) slice,
    energy = ifft2( fft2(Q) @ fft2(K)^T * dk ).real
Because the DFT matrix F satisfies F @ F^T = n * P (P = index-negation
permutation), this collapses EXACTLY to
    energy = dk * D * Q @ K~^T        with K~[j, d] = K[j, (-d) mod D]
i.e. plain attention with K's head-dim index flipped (mod D) and an extra
scale of dk * D = 512.  No FFTs are needed; the flip and scale are folded
into host-side slices of the Wk / Wq projection weights.

Sharding: 8 cores = 4 batches x 2 head-groups (4 heads each).  Each core
gets q[b]^T, kv[b]^T (pre-transposed on host so the contraction dim lands
on SBUF partitions) plus its slice of the projection weights, computes
attention for its 4 heads, and applies its slice of Wo.  The host sums the
two partial Wo products per batch (the unshard-reduce) in fp32.

Precision scheme (PE fp32 matmuls are 4 cyc/row; fp16 is 1 cyc/row):
every value on the logit path is split hi/lo into two fp16 parts
(x = xh + xl, products of fp16 are exact in the fp32 PSUM accumulator), so
  x @ y ~= xh@yh + (xh@yl + xl@yh)     [~22-bit mantissa, err ~1e-6 rel]
One extra all-ones row in the stationary K operand times a "-rowmax" row
in the moving Q operand injects the softmax max-subtraction bias directly
into the S^T matmul.  The row max comes from a separate hi-only fp16 pass
(error +-15 absolute on ~25000-scale logits is harmless: the softmax
normalization cancels any bias error, it only has to keep exp() in range).

Schedule (v2, engine-balance + HAM-warmth rework of the 200us baseline):
  - per-head steady state is paced by DVE (8 reduce_max) / ACT (8 exp)
    / PE (48 matmuls) all at ~10-11us; everything else is moved off the
    critical engines:
      * row-max bounce: fp16 colmax -> DRAM -> DMA straight into the qm
        bias row (no ACT copy, no f32->f16 cast op).
      * softmax denominators: rowsum row -> DRAM bounce into (128,8)
        layout -> DVE reciprocal (127ns) -> bounce back -> gpsimd
        partition_broadcast (the ONLY gpsimd op stream, so its custom
        library loads once -- the baseline re-loaded it every head and
        ate a 7.9us gpsimd DRAIN on the tail).  The last head uses an
        ACT ln/exp chain instead (ACT is idle at the tail and the chain
        is 2 DMAs shorter).
      * recip/mul of head h are EMITTED a head late so the strict-FIFO
        DVE queue never parks on a DMA wait in front of the next head's
        reduce_max stream.
  - maxpass(h+2) matmul blocks are interleaved into mainpass(h)'s j-loop
    so PE never idles (HAM stays at K=8/8 = 2.4 GHz; the baseline spent
    108us of 200 at 1.2 GHz).
  - input tensors are DMA'd in two half-T pieces so the first projection
    matmuls start ~2us earlier.
  - output partials are bf16 (halves the tail DMA); host sums in fp32.
"""

import numpy as np
from contextlib import ExitStack

import concourse.bass as bass
import concourse.tile as tile
from concourse import bacc, mybir
from concourse.bass_utils import run_bass_kernel_spmd

F32 = mybir.dt.float32
F16 = mybir.dt.float16
BF16 = mybir.dt.bfloat16
AX = mybir.AxisListType
AF = mybir.ActivationFunctionType

T = 1024          # sequence length
E = 512           # embed dim
H = 8             # total heads
D = E // H        # head dim = 64
NH = 4            # heads per core
DX = NH * (D + 1) # vp columns incl. ones = 260
N_CORES = 8
SCALE = float(D) * float(D) ** 0.5  # dk * D = 512.0

TRACE = False          # set by test harness; adds NTFF profiling
DEBUG = False          # adds debug tap outputs (sim debugging only)
LAST_EXEC_NS = None


def _emit(ctx, tc, dram):
    nc = tc.nc
    const = ctx.enter_context(tc.tile_pool(name="const", bufs=1))
    ps_big = ctx.enter_context(tc.tile_pool(name="ps_big", bufs=2, space="PSUM"))
    ps_av = ctx.enter_context(tc.tile_pool(name="ps_av", bufs=1, space="PSUM"))
    ps_sm = ctx.enter_context(tc.tile_pool(name="ps_sm", bufs=2, space="PSUM"))
    atp = ctx.enter_context(tc.tile_pool(name="atp", bufs=4))
    outp = ctx.enter_context(tc.tile_pool(name="outp", bufs=8))
    dramp = ctx.enter_context(tc.tile_pool(name="dramp", bufs=1, space="DRAM"))

    # ---- input loads (all fp16 on the wire) ----
    def load_w(name, cols):
        t3 = const.tile([128, 4, cols], F16, tag=name, name=name)
        nc.sync.dma_start(
            t3[:], dram[name][:].rearrange("(c p) t -> p c t", p=128)
        )
        return [t3[:, e, :] for e in range(4)]

    def load_x(name):
        # two half-T DMAs: the n=0 projection matmuls only wait on half 1
        t3 = const.tile([128, 4, T], F16, tag=name, name=name)
        src = dram[name][:].rearrange("(c p) t -> p c t", p=128)
        nc.sync.dma_start(t3[:, :, 0:512], src[:, :, 0:512])
        nc.sync.dma_start(t3[:, :, 512:1024], src[:, :, 512:1024])
        return [t3[:, e, :] for e in range(4)]

    # load order matters: the first projection matmuls need wqh+ql first
    wqh = load_w("wqh", NH * D)
    ql_in = load_x("ql")
    wql = load_w("wql", NH * D)
    qh_in = load_x("qh")
    wkh = load_w("wkh", NH * D)
    kvl_in = load_x("kvl")
    wkl = load_w("wkl", NH * D)
    kvh_in = load_x("kvh")
    wv = load_w("wv", DX)
    wo3 = const.tile([128, 2, E], F16, tag="wo", name="wo")
    nc.sync.dma_start(
        wo3[:], dram["wo"][:].rearrange("(g p) t -> p g t", p=128)
    )
    wo = [wo3[:, g, :] for g in range(2)]

    # PE warm-up: dummy matmuls fill the input-DMA window so the HAM clock
    # gate is already at 8/8 (2.4 GHz) when the projections start.
    wrm = const.tile([128, 512], F16, tag="wrm", name="wrm")
    nc.vector.memset(wrm[:], 0.0)
    for w in range(12):
        pw = ps_sm.tile([128, E], F32, tag="sm", name="psw")
        nc.tensor.matmul(pw[:], lhsT=wrm[:, 0:128], rhs=wrm[:],
                         start=True, stop=True)

    # ---- hi/lo projections ----
    # per head: qm (65, T) fp16 = [qp_hi; -rowmax (DMA'd in later)]
    #           km (65, T) fp16 = [kp_hi; ones]
    #           qc (128, T) fp16 = [qp_lo; qp_hi]   (cross moving operand)
    #           kc (128, T) fp16 = [kp_hi; kp_lo]   (cross stationary)
    qm = [const.tile([65, T], F16, tag=f"qm{h}", name=f"qm{h}") for h in range(NH)]
    km = [const.tile([65, T], F16, tag=f"km{h}", name=f"km{h}") for h in range(NH)]
    qc = [const.tile([128, T], F16, tag=f"qc{h}", name=f"qc{h}") for h in range(NH)]
    kc = [const.tile([128, T], F16, tag=f"kc{h}", name=f"kc{h}") for h in range(NH)]

    def proj(wh, wl, xh, xl, dm, dc, hi_row, m):
        msl = slice(m * 128, (m + 1) * 128)
        ps = ps_big.tile([128, T], F32, tag="big", name="psb")
        for n in range(2):
            nsl = slice(n * 512, (n + 1) * 512)
            mms = (
                [(wh[e], xl[e]) for e in range(4)]      # cross: Wh @ xl
                + [(wl[e], xh[e]) for e in range(4)]    # cross: Wl @ xh
                + [(wh[e], xh[e]) for e in range(4)]    # main:  Wh @ xh
            )
            for i_mm, (lw, rx) in enumerate(mms):
                nc.tensor.matmul(
                    ps[:, nsl],
                    lhsT=lw[:, msl],
                    rhs=rx[:, nsl],
                    start=(i_mm == 0), stop=(i_mm == len(mms) - 1),
                )
        for hh in range(2):
            h = 2 * m + hh
            psl = slice(hh * 64, hh * 64 + 64)
            lo_row = 64 - hi_row
            # hi part (fp16 cast) into the K=65 "main" tile
            nc.scalar.copy(dm[h][0:64, :], ps[psl, :])
            # hi copy into the cross tile: cheap fp16 SBUF->SBUF; split
            # between DVE (4x mode) and ACT to balance the drain
            if hh == 0:
                nc.vector.tensor_copy(dc[h][hi_row:hi_row + 64, :], dm[h][0:64, :])
            else:
                nc.scalar.copy(dc[h][hi_row:hi_row + 64, :], dm[h][0:64, :])
            # lo part = ps - hi (fp16)
            nc.vector.tensor_sub(dc[h][lo_row:lo_row + 64, :], ps[psl, :],
                                 dm[h][0:64, :])

    # ---- per-head attention pieces ----
    colmax = [None] * NH

    def max_block(h, i):
        ps = ps_big.tile([128, T], F32, tag="big", name="psb")
        for n in range(2):
            nsl = slice(n * 512, (n + 1) * 512)
            nc.tensor.matmul(
                ps[:, nsl],
                lhsT=qm[h][0:64, i * 128:(i + 1) * 128],
                rhs=km[h][0:64, nsl],
                start=True, stop=True,
            )
        nc.vector.reduce_max(colmax[h][:, i:i + 1], ps[:], axis=AX.X,
                             negate=True)

    def max_bounce(h):
        # (128, 8) fp16 -> DRAM -> straight into qm's bias row (1, 1024)
        sc = dramp.tile([8, 128], F16, tag=f"sc{h}", name=f"sc{h}")
        nc.sync.dma_start(sc[:].rearrange("c p -> p c"), colmax[h][:])
        nc.sync.dma_start(qm[h][64:65, :], sc[:].rearrange("c p -> (c p)"))

    def maxpass(h):
        colmax[h] = const.tile([128, 8], F16, tag=f"cm{h}", name=f"cm{h}")
        for i in range(8):
            max_block(h, i)
        max_bounce(h)

    yun = [None] * NH
    rs = [None] * NH
    rcp = [None] * NH
    yh = [const.tile([128, T], F16, tag=f"yh{g}", name=f"yh{g}") for g in range(2)]

    def mainpass(h, hmax=None):
        # main pass: S^T - max = cross + main(bias) matmuls, exp, AV.
        # hmax: head whose hi-only max pass is interleaved block-by-block.
        if hmax is not None:
            colmax[hmax] = const.tile([128, 8], F16, tag=f"cm{hmax}",
                                      name=f"cm{hmax}")
        oex = ps_av.tile([65, T], F32, tag="av", name="oex")
        for j in range(8):
            jsl = slice(j * 128, (j + 1) * 128)
            ps = ps_big.tile([128, T], F32, tag="big", name="psb")
            for n in range(2):
                nsl = slice(n * 512, (n + 1) * 512)
                nc.tensor.matmul(
                    ps[:, nsl], lhsT=kc[h][:, jsl], rhs=qc[h][:, nsl],
                    start=True, stop=False,
                )
                nc.tensor.matmul(
                    ps[:, nsl], lhsT=km[h][:, jsl], rhs=qm[h][:, nsl],
                    start=False, stop=True,
                )
            at = atp.tile([128, T], BF16, tag="at", name="at")
            nc.scalar.activation(at[:], ps[:], AF.Exp)
            for n in range(2):
                nsl = slice(n * 512, (n + 1) * 512)
                nc.tensor.matmul(
                    oex[:, nsl],
                    lhsT=vpx[j][:, h, :],
                    rhs=at[:, nsl],
                    start=(j == 0), stop=(j == 7),
                )
            if hmax is not None:
                max_block(hmax, j)
        if hmax is not None:
            max_bounce(hmax)

        # Evacuate PSUM (frees the oex slot for the next head).
        yun[h] = const.tile([64, T], F32, tag=f"yun{h}", name=f"yun{h}")
        nc.scalar.copy(yun[h][:], oex[0:64, :])
        if h < 3:
            # rowsums out to SBUF, then DRAM-bounce into (128, 8) layout
            # for a cheap DVE reciprocal (emitted a head later).
            rs[h] = const.tile([1, T], F32, tag=f"rs{h}", name=f"rs{h}")
            nc.vector.tensor_copy(rs[h][:], oex[64:65, :])
            rdf = dramp.tile([T], F32, tag=f"rdf{h}", name=f"rdf{h}")
            nc.sync.dma_start(rdf[:].rearrange("(p t) -> p t", p=1), rs[h][:])
            rsT = const.tile([128, 8], F32, tag=f"rsT{h}", name=f"rsT{h}")
            nc.sync.dma_start(rsT[:], rdf[:].rearrange("(b p) -> p b", p=128))
            rs[h] = rsT
        else:
            # tail head: ACT is idle here and the ln/exp chain is shorter.
            lns = const.tile([1, T], F32, tag="ln3", name="ln3")
            nc.scalar.activation(lns[:], oex[64:65, :], AF.Ln)
            rcp[h] = const.tile([1, T], F32, tag="rcp3", name="rcp3")
            nc.scalar.activation(rcp[h][:], lns[:], AF.Exp, scale=-1.0)

    def finish_norm(h):
        # reciprocal + broadcast + normalize-multiply for head h.  Emitted
        # one head late so the DVE FIFO never parks on the DRAM bounce.
        if h < 3:
            rcpT = const.tile([128, 8], F32, tag=f"rcpT{h}", name=f"rcpT{h}")
            nc.vector.reciprocal(rcpT[:], rs[h][:])
            rdb = dramp.tile([T], F32, tag=f"rdb{h}", name=f"rdb{h}")
            nc.sync.dma_start(rdb[:].rearrange("(b p) -> p b", p=128), rcpT[:])
            rcp[h] = const.tile([1, T], F32, tag=f"rcp{h}", name=f"rcp{h}")
            nc.sync.dma_start(rcp[h][:], rdb[:].rearrange("(p t) -> p t", p=1))
        recb = const.tile([64, T], F32, tag=f"rcb{h}", name=f"rcb{h}")
        nc.gpsimd.partition_broadcast(recb[:], rcp[h][:])
        g, half = divmod(h, 2)
        nc.vector.tensor_mul(
            yh[g][half * 64:(half + 1) * 64, :], yun[h][:], recb[:]
        )

    # ---- output projection, split by head pair ----
    ot = [outp.tile([128, E], BF16, tag="ot", name=f"ot{i}") for i in range(8)]

    def wo_pass(g):
        for i in range(8):
            pso = ps_sm.tile([128, E], F32, tag="sm", name="pso")
            nc.tensor.matmul(
                pso[:],
                lhsT=yh[g][:, i * 128:(i + 1) * 128],
                rhs=wo[g][:],
                start=True, stop=True,
            )
            if g == 0:
                nc.vector.tensor_copy(ot[i][:], pso[:])
            else:
                nc.vector.tensor_add(ot[i][:], ot[i][:], pso[:])
                nc.sync.dma_start(dram["out"][i * 128:(i + 1) * 128, :], ot[i][:])

    # ---- V projection (t-major bf16, + ones column per head) ----
    vpx = [const.tile([128, NH, D + 1], BF16, tag=f"vpx{t}", name=f"vpx{t}")
           for t in range(8)]

    def vpx_block():
        for t in range(8):
            ps = ps_sm.tile([128, E], F32, tag="sm", name="pss")
            for e in range(4):
                nc.tensor.matmul(
                    ps[:, 0:DX],
                    lhsT=kvh_in[e][:, t * 128:(t + 1) * 128],
                    rhs=wv[e][:],
                    start=(e == 0), stop=(e == 3),
                )
            nc.scalar.copy(vpx[t][:].rearrange("p h d -> p (h d)"), ps[:, 0:DX])
            nc.vector.memset(vpx[t][:, :, D:D + 1], 1.0)

    # ---- emission schedule ----
    proj(wqh, wql, qh_in, ql_in, qm, qc, 64, m=0)   # heads 0,1 q
    proj(wqh, wql, qh_in, ql_in, qm, qc, 64, m=1)   # heads 2,3 q
    proj(wkh, wkl, kvh_in, kvl_in, km, kc, 0, m=0)  # heads 0,1 k
    for h in range(NH):
        nc.vector.memset(km[h][64:65, :], 1.0)
    maxpass(0)
    proj(wkh, wkl, kvh_in, kvl_in, km, kc, 0, m=1)  # heads 2,3 k
    vpx_block()
    maxpass(1)
    mainpass(0, hmax=2)
    mainpass(1, hmax=3)
    finish_norm(0)
    mainpass(2)
    finish_norm(1)
    wo_pass(0)
    mainpass(3)
    finish_norm(2)
    finish_norm(3)
    wo_pass(1)

    if DEBUG:
        nc.sync.dma_start(dram["d_qmrow"][:], qm[0][64:65, :])
        nc.sync.dma_start(dram["d_colmax"][:], colmax[0][:])
        nc.sync.dma_start(dram["d_rsT"][:], rs[0][:])
        nc.sync.dma_start(dram["d_rcp"][:], rcp[0][:])
        nc.sync.dma_start(dram["d_yun"][:], yun[0][:])
        nc.sync.dma_start(dram["d_yh"][:], yh[0][:])
        nc.sync.dma_start(dram["d_qm"][:], qm[0][0:64, :])
        nc.sync.dma_start(dram["d_km"][:], km[0][:])


class _Bacc(bacc.Bacc):
    """Bacc whose activation-table chooser can only pick the combined
    natural_log_exp_and_others set for Exp/Ln: the exp-only sets are
    blanked (entries kept so act_func_set_id indices stay aligned with
    act_info.json), which removes the per-head Exp<->Ln table swap
    (~2.7us each)."""

    def insert_act_table_loads(self):
        import bass_rust as _bass_rust
        from concourse.hw_specs import get_activation_tables
        has_activation = any(
            isinstance(i, mybir.InstActivation)
            for b in self.main_func.blocks
            for i in b.instructions
        )
        if not has_activation:
            return
        tables = []
        for name, fns in get_activation_tables(self.m.arch).items():
            if name in ("exp_and_others", "exp_and_friends"):
                fns = set()
            tables.append((name, fns))
        _bass_rust.insert_act_table_loads(self, tables)


def build_program():
    # Bacc (not raw Bass): its compile() splits multi-sem matmul waits onto
    # ldweights (TRN2 allows 1 wait/instruction), auto-inserts gpsimd
    # library loads for PartitionBroadcast, and lowers extended-ISA bytes.
    nc = _Bacc("TRN2", target_bir_lowering=False, debug=False)
    dp = nc.declare_dram_parameter
    dram = {}
    for name in ("qh", "ql", "kvh", "kvl"):
        dram[name] = dp(name, [E, T], F16, isOutput=False)
    for name in ("wqh", "wql", "wkh", "wkl"):
        dram[name] = dp(name, [E, NH * D], F16, isOutput=False)
    dram["wv"] = dp("wv", [E, DX], F16, isOutput=False)
    dram["wo"] = dp("wo", [NH * D, E], F16, isOutput=False)
    dram["out"] = dp("out", [T, E], BF16, isOutput=True)
    if DEBUG:
        dram["d_qmrow"] = dp("d_qmrow", [1, T], F16, isOutput=True)
        dram["d_colmax"] = dp("d_colmax", [128, 8], F16, isOutput=True)
        dram["d_rsT"] = dp("d_rsT", [128, 8], F32, isOutput=True)
        dram["d_rcp"] = dp("d_rcp", [1, T], F32, isOutput=True)
        dram["d_yun"] = dp("d_yun", [64, T], F32, isOutput=True)
        dram["d_yh"] = dp("d_yh", [128, T], F16, isOutput=True)
        dram["d_qm"] = dp("d_qm", [64, T], F16, isOutput=True)
        dram["d_km"] = dp("d_km", [65, T], F16, isOutput=True)
    with ExitStack() as ctx:
        tc = ctx.enter_context(tile.TileContext(nc))
        _emit(ctx, tc, dram)
    nc.finalize()  # Bacc.finalize runs compile() then freezes
    return nc


_PROGRAM = None


def _get_program():
    global _PROGRAM
    if _PROGRAM is None:
        _PROGRAM = build_program()
    return _PROGRAM


def _split16(x):
    h = x.astype(np.float16)
    l = (x - h.astype(np.float32)).astype(np.float16)
    return h, l


def make_in_maps(q, kv, Wq, Wk, Wv, Wo):
    in_maps = []
    for c in range(N_CORES):
        b, g = divmod(c, 2)
        heads = [g * NH + j for j in range(NH)]
        idx_q = [d * H + h for h in heads for d in range(D)]
        idx_k = [((D - d) % D) * H + h for h in heads for d in range(D)]
        qTh, qTl = _split16(np.ascontiguousarray(q[b].T))
        kvTh, kvTl = _split16(np.ascontiguousarray(kv[b].T))
        wq_h, wq_l = _split16(Wq[:, idx_q] * np.float32(SCALE))
        wk_h, wk_l = _split16(Wk[:, idx_k])
        wv_c = np.zeros((E, DX), np.float16)
        for j, h in enumerate(heads):
            wv_c[:, j * (D + 1):j * (D + 1) + D] = \
                Wv[:, [d * H + h for d in range(D)]].astype(np.float16)
        in_maps.append({
            "qh": qTh, "ql": qTl, "kvh": kvTh, "kvl": kvTl,
            "wqh": wq_h, "wql": wq_l, "wkh": wk_h, "wkl": wk_l,
            "wv": wv_c,
            "wo": Wo[g * NH * D:(g + 1) * NH * D, :].astype(np.float16),
        })
    return in_maps


def kernel(**inputs):
    global LAST_EXEC_NS
    q = np.asarray(inputs["q"], dtype=np.float32)
    kv = np.asarray(inputs["kv"], dtype=np.float32)
    Wq = np.asarray(inputs["Wq"], dtype=np.float32)
    Wk = np.asarray(inputs["Wk"], dtype=np.float32)
    Wv = np.asarray(inputs["Wv"], dtype=np.float32)
    Wo = np.asarray(inputs["Wo"], dtype=np.float32)
    B = q.shape[0]

    nc = _get_program()
    in_maps = make_in_maps(q, kv, Wq, Wk, Wv, Wo)
    res = run_bass_kernel_spmd(nc, in_maps, list(range(N_CORES)), trace=TRACE)
    LAST_EXEC_NS = res.exec_time_ns

    out = np.empty((B, T, E), np.float32)
    for b in range(B):
        out[b] = (np.asarray(res.results[2 * b]["out"], dtype=np.float32)
                  + np.asarray(res.results[2 * b + 1]["out"], dtype=np.float32))
    return out
